# revision 36
# baseline (speedup 1.0000x reference)
"""CrossModalAttention Trainium2 kernel.

Full inputs -> full output. Internally: 8-way SPMD over (batch, key-half):
core = 2*b + h owns keys [h*2048, (h+1)*2048) of batch b and computes the
UNNORMALIZED attention output over those keys for ALL 4096 queries, plus
the per-query partition sum Z. The host sums the two partials per batch
and normalizes.

Math (per batch), with x = concat(img, label, z) [C=256, N=4096]:
  q = wq x + bq, k = wk x (bk dropped: constant-in-key terms cancel in
  softmax), v = wv x + bv
  S[n, m] = q_n . k_m = x_n^T A x_m + t_m       A = wq^T wk,  t = (wk^T bq)^T x
so the Q projection never happens on device: the kernel computes
kk = A x once (keys only), streams raw x as the query operand, and t - SHIFT
rides in as the per-partition bias of the exp activation (scores are
computed transposed, keys on partitions). A and t come from the host.

Layouts (all per core, m = this core's 2048 keys after rotation):
  ST[m, n] via lhsT = kk chunk [c,128], rhs = x [c, 512]   (zero transposes)
  P = exp(ST + (t[m] - SHIFT))  -> bf16 pt
  out[n, c] = sum_m pt[m, n] * vT[m, c]; vT has ones columns appended so
  the same accumulation yields Z[n]. Raw (out|Z) goes to HBM; the host
  divides by Z (summing the two key-halves first) and adds bv.

Schedule notes:
- ~20 warmup matmuls on a zeroed tile keep the PE busy from the end of
  the framework preamble so the HAM clock-gate reaches 2.4 GHz before the
  projections start (otherwise the first ~17 us run at 1.2 GHz).
- Input DMAs are issued from three queues (sync: weights, gpsimd/scalar:
  the two x channel-halves) with small leading pieces, so the first
  projection matmul starts ~8 us in instead of ~14.
- Score PSUM tiles are [128,512] (1 bank) with bufs=6: the exp (ACT) is
  slightly slower per tile than the 2 matmuls that fill it, and a deep
  rotation absorbs the drift without stalling the PE.
- Block interleave [ST0][ST1][PV0][ST2][PV1]...: PV(nb) starts a full
  score block after ST(nb), so exp(nb) is always done; pt is
  double-buffered. The last PV block interleaves its first two
  sub-blocks to cover the final exp tail.
- Rotation trick: core h's keys are host-rotated to columns 0..2047, so
  the SPMD program is identical across cores. Output rows are queries in
  rotated order; the host un-rotates.
- SHIFT=85 as in the proven baseline: scores lie in [-128, 132], exp in
  fp32 range with margin. Partial sums stay finite in fp32 (Z <= 2048*e^47).
"""

import numpy as np

import concourse.bacc as bacc
import concourse.mybir as mybir
import concourse.tile as tile
from concourse import bass_utils

B = 4
C = 256  # channels after concat
H = W = 64
N = H * W  # 4096 pixels
NCORES = 8
MHALF = N // 2  # 2048 keys per core
SHIFT = 85.0

F32 = mybir.dt.float32
F32R = mybir.dt.float32r
BF16 = mybir.dt.bfloat16

FQ = 512  # query-block free dim
NB = N // FQ  # 8 query blocks per core (all queries)
MJ = MHALF // 128  # 16 key chunks of 128
CA = C + 2  # channels + ones col + pad (even free dim for the PE)
CZ = C + 1  # shipped columns: values + Z


def _emit(nc, tc, x_d, wp_d, tb_d, out_d):
    f32 = F32
    f32r = F32R
    mm = nc.tensor.matmul
    Exp = mybir.ActivationFunctionType.Exp
    Copy = mybir.ActivationFunctionType.Copy

    with tc.tile_pool(name="consts", bufs=1) as cp, \
         tc.tile_pool(name="proj", bufs=1) as pp, \
         tc.tile_pool(name="xp", bufs=1) as xp, \
         tc.tile_pool(name="bigps", bufs=6, space="PSUM") as bigps, \
         tc.tile_pool(name="attn", bufs=2) as app, \
         tc.tile_pool(name="ob", bufs=3) as op, \
         tc.tile_pool(name="vps", bufs=2, space="PSUM") as vps:
        wp = [cp.tile([128, 512], f32r, name=f"wp{i}", tag=f"wp{i}")
              for i in range(2)]
        tb = cp.tile([128, MJ], f32, name="tb", tag="tb")
        ones32 = cp.tile([128, 2 * MJ], f32, name="ones32", tag="ones32")
        warm = cp.tile([128, 512], f32, name="warm", tag="warm")
        nc.vector.memset(warm[:], 0.0)
        nc.vector.memset(ones32[:], 1.0)

        kk = [pp.tile([128, MHALF], f32r, name=f"kk{i}", tag=f"kk{i}")
              for i in range(2)]
        vT = pp.tile([128, MJ * CA], BF16, name="vT", tag="vT")
        vT3 = vT.rearrange("p (b c) -> p b c", c=CA)
        nc.vector.tensor_copy(
            vT3[:, :, C:C + 2],
            ones32[:].rearrange("p (b c) -> p b c", c=2))

        x_sb = [xp.tile([128, N], f32r, name=f"x{i}", tag=f"x{i}")
                for i in range(2)]

        # ---- DMAs: three queues in parallel. The phase0-critical span is
        # x[:, 0:2048] on both channel halves + the weights (2.5 MB): the
        # [1536:2048] pieces ride on the sync queue after the weights so
        # all three queues carry ~equal critical bytes.
        # DMA order is matched to the phase0/ST0 cascade's consumption:
        # segment bi consumes x[:, bi*256:(bi+1)*256] of both channel
        # halves. The first kk matmul is gated by only tb + the four A
        # quarter-tiles + x[:, 0:256] x2 (~450 KB over three queues).
        nc.sync.dma_start(tb[:], tb_d.ap()[:, :])
        nc.sync.dma_start(wp[0][:, 0:128], wp_d.ap()[0:128, 0:128])
        nc.sync.dma_start(wp[0][:, 128:256], wp_d.ap()[0:128, 128:256])
        nc.sync.dma_start(wp[0][:, 256:512], wp_d.ap()[0:128, 256:512])
        nc.sync.dma_start(x_sb[1][:, 512:1024], x_d.ap()[128:256, 512:1024])
        nc.sync.dma_start(x_sb[1][:, 2048:3072], x_d.ap()[128:256, 2048:3072])
        nc.sync.dma_start(x_sb[1][:, 3072:4096], x_d.ap()[128:256, 3072:4096])
        nc.gpsimd.dma_start(wp[1][:, 0:128], wp_d.ap()[128:256, 0:128])
        for s, e in [(0, 256), (256, 512), (512, 1024), (1024, 1536),
                     (1536, 2048), (2048, 3072), (3072, 4096)]:
            nc.gpsimd.dma_start(x_sb[0][:, s:e], x_d.ap()[0:128, s:e])
        nc.scalar.dma_start(x_sb[1][:, 0:256], x_d.ap()[128:256, 0:256])
        nc.scalar.dma_start(wp[1][:, 128:256], wp_d.ap()[128:256, 128:256])
        nc.scalar.dma_start(wp[1][:, 256:512], wp_d.ap()[128:256, 256:512])
        nc.scalar.dma_start(x_sb[1][:, 256:512], x_d.ap()[128:256, 256:512])
        nc.scalar.dma_start(x_sb[1][:, 1024:1536], x_d.ap()[128:256, 1024:1536])
        nc.scalar.dma_start(x_sb[1][:, 1536:2048], x_d.ap()[128:256, 1536:2048])

        # ---- PE warmup (HAM un-throttle) ----
        # Warmup: keep the PE busy from the end of the framework preamble
        # until the first x pieces land (~12 us), so the HAM clock-gate
        # sees one continuous busy window and switches to 2.4 GHz early.
        wps = bigps.tile([128, 512], f32, name="wps", tag="ps")
        for _ in range(5):
            mm(wps[:], warm[:, 0:128], warm[:], start=True, stop=True)

        # ---- phase 0: kk = A x, vT = x^T wv^T  (keys 0..2047) ----
        # segments sized to the x pieces so compute unblocks as they land
        def kk_block(s, w):
            ps = bigps.tile([128, 512], f32, name="ps", tag="ps")
            if w == 256:
                for co in range(2):
                    for ci in range(2):
                        mm(ps[:, co * 256:(co + 1) * 256],
                           wp[ci][:, co * 128:(co + 1) * 128],
                           x_sb[ci][:, s:s + 256],
                           start=ci == 0, stop=ci == 1)
                for co in range(2):
                    nc.vector.tensor_copy(kk[co][:, s:s + 256],
                                          ps[:, co * 256:(co + 1) * 256])
                return
            ps2 = bigps.tile([128, 512], f32, name="ps", tag="ps")
            for co, pt_ in ((0, ps), (1, ps2)):
                for ci in range(2):
                    mm(pt_[:], wp[ci][:, co * 128:(co + 1) * 128],
                       x_sb[ci][:, s:s + 512], start=ci == 0, stop=ci == 1)
                nc.vector.tensor_copy(kk[co][:, s:s + 512], pt_[:])

        def v_block(g):  # g covers key chunks 2g, 2g+1
            ps = bigps.tile([128, 512], f32, name="ps", tag="ps")
            for j in range(2):
                mj = g * 2 + j
                for ci in range(2):
                    mm(ps[:, j * 256:(j + 1) * 256],
                       x_sb[ci][:, mj * 128:(mj + 1) * 128],
                       wp[ci][:, 256:512], start=ci == 0, stop=ci == 1)
            dst = vT3[:, g * 2:(g + 1) * 2, 0:C]
            src = ps[:].rearrange("p (b c) -> p b c", c=256)
            if g < 4:
                nc.vector.tensor_copy(dst, src)
            else:
                nc.scalar.activation(dst, src, Copy)

        # ---- attention ----
        def st_range(nb, ptb, mja, mjb):
            for mj in range(mja, mjb):
                ps = bigps.tile([128, 512], f32, name="st", tag="ps")
                for ci in range(2):
                    mm(ps[:], kk[ci][:, mj * 128:(mj + 1) * 128],
                       x_sb[ci][:, nb * FQ:(nb + 1) * FQ],
                       start=ci == 0, stop=ci == 1)
                nc.scalar.activation(
                    ptb[:, mj * FQ:(mj + 1) * FQ], ps[:], Exp,
                    bias=tb[:, mj:mj + 1])

        # phase0/ST0 cascade, piece-major: each x piece feeds its kk
        # segment + v-groups, which immediately unblock the matching ST0
        # key chunks — the PE never idles long on the x DMA and the HAM
        # clock-gate warms during the input stream.
        pt0 = app.tile([128, MJ * FQ], BF16, name="pt", tag="pt")
        SEGS = [(0, 256), (256, 256), (512, 512), (1024, 512), (1536, 512)]
        for s, w in SEGS:
            kk_block(s, w)
            for g in range(s // 256, (s + w) // 256):
                v_block(g)
            st_range(0, pt0, s // 128, (s + w) // 128)

        def pv_mm(po, ptb, ns, mj, start, stop):
            o = mj * FQ + ns * 128
            mm(po[:], ptb[:, o:o + 128], vT[:, mj * CA:(mj + 1) * CA],
               start=start, stop=stop)

        # Output rides as bf16 (the DMA-ring write path is the scarce
        # resource). The fp32 Z is bit-split across the last two bf16
        # columns (exact — the host reassembles the fp32 bits).
        def pv_finish(po, nb, ns, eng=None):
            ob = op.tile([128, CA], BF16, name="ob", tag="ob")
            nc.vector.tensor_copy(ob[:, 0:C], po[:, 0:C])
            nc.vector.tensor_copy(ob[:, C:C + 2].bitcast(F32),
                                  po[:, C:C + 1])
            r = nb * FQ + ns * 128
            if eng is None:
                eng = nc.gpsimd if ns % 2 == 0 else nc.sync
            eng.dma_start(out_d.ap()[r:r + 128, :], ob[:])

        def pv_block(nb, ptb, last):
            if not last:
                for ns in range(4):
                    po = vps.tile([128, CA], f32, name="pv", tag="pv")
                    for mj in range(MJ):
                        pv_mm(po, ptb, ns, mj, mj == 0, mj == MJ - 1)
                    pv_finish(po, nb, ns)
                return
            # last block: interleave the first two sub-blocks so the
            # accumulation never waits on the trailing exp chunks.
            po0 = vps.tile([128, CA], f32, name="pv0", tag="pv")
            for mj in range(12):
                pv_mm(po0, ptb, 0, mj, mj == 0, False)
            po1 = vps.tile([128, CA], f32, name="pv1", tag="pv")
            for mj in range(8):
                pv_mm(po1, ptb, 1, mj, mj == 0, False)
            # final-block tail: all DMAs ride the scalar queue (its ring
            # is empty by now — the gpsimd/sync rings still carry earlier
            # blocks), and the last sub-block's Z hi/lo chain runs on
            # scalar in parallel with the bulk copy on vector.
            for mj in range(12, MJ):
                pv_mm(po0, ptb, 0, mj, False, mj == MJ - 1)
            pv_finish(po0, nb, 0, nc.scalar)
            for mj in range(8, MJ):
                pv_mm(po1, ptb, 1, mj, False, mj == MJ - 1)
            pv_finish(po1, nb, 1, nc.scalar)
            po2 = vps.tile([128, CA], f32, name="pv", tag="pv")
            for mj in range(MJ):
                pv_mm(po2, ptb, 2, mj, mj == 0, mj == MJ - 1)
            pv_finish(po2, nb, 2, nc.scalar)
            po3 = vps.tile([128, CA], f32, name="pv", tag="pv")
            for mj in range(MJ):
                pv_mm(po3, ptb, 3, mj, mj == 0, mj == MJ - 1)
            ob = op.tile([128, CA], BF16, name="ob", tag="ob")
            nc.scalar.activation(ob[:, 0:C], po3[:, 0:C], Copy)
            nc.vector.tensor_copy(ob[:, C:C + 2].bitcast(F32),
                                  po3[:, C:C + 1])
            r = nb * FQ + 3 * 128
            nc.scalar.dma_start(out_d.ap()[r:r + 128, :], ob[:])

        pts = [pt0]
        for nb in range(1, NB):
            ptb = app.tile([128, MJ * FQ], BF16, name="pt", tag="pt")
            pts.append(ptb)
            st_range(nb, ptb, 0, MJ)
            pv_block(nb - 1, pts[nb - 1], False)
        pv_block(NB - 1, pts[NB - 1], True)


_CACHE = {}


def _build():
    if "nc" in _CACHE:
        return _CACHE["nc"]
    nc = bacc.Bacc("TRN2", target_bir_lowering=False, debug=False)
    x_d = nc.dram_tensor("x", [C, N], F32R, kind="ExternalInput")
    wp_d = nc.dram_tensor("wp", [C, 512], F32R, kind="ExternalInput")
    tb_d = nc.dram_tensor("tb", [128, MJ], F32, kind="ExternalInput")
    out_d = nc.dram_tensor("out", [N, CA], BF16, kind="ExternalOutput")
    with tile.TileContext(nc) as tc:
        _emit(nc, tc, x_d, wp_d, tb_d, out_d)
    nc.compile()
    _CACHE["nc"] = nc
    return nc


def _in_maps(img, label, z, wq, bq, wk, bk, wv, bv):
    x = np.concatenate(
        [np.asarray(img), np.asarray(label), np.asarray(z)], axis=1
    ).reshape(B, C, N).astype(np.float32)
    wq64 = np.asarray(wq, np.float64)
    wk64 = np.asarray(wk, np.float64)
    AT = (wk64.T @ wq64).astype(np.float32)  # lhsT for kk = A x, A = wq^T wk
    wvT = np.ascontiguousarray(np.asarray(wv).T, np.float32)
    wp = np.concatenate([AT, wvT], axis=1)  # [256, 512]
    u = (wk64.T @ np.asarray(bq, np.float64)).astype(np.float64)  # [256]
    maps = []
    for core in range(NCORES):
        b, h = divmod(core, 2)
        # rotate so this core's keys are columns 0..MHALF-1
        xc = x[b] if h == 0 else np.ascontiguousarray(
            np.concatenate([x[b][:, MHALF:], x[b][:, :MHALF]], axis=1))
        t = (u @ xc[:, :MHALF].astype(np.float64)).astype(np.float32)
        tbv = np.ascontiguousarray(t.reshape(MJ, 128).T) - np.float32(SHIFT)
        maps.append({"x": xc, "wp": wp, "tb": tbv})
    return maps


def kernel(img, label, z, wq, bq, wk, bk, wv, bv):
    nc = _build()
    maps = _in_maps(img, label, z, wq, bq, wk, bk, wv, bv)
    res = bass_utils.run_bass_kernel_spmd(nc, maps,
                                          core_ids=list(range(NCORES)))
    def _split(raw):
        # vals in bf16; Z arrives as raw fp32 bits spread over the last
        # two bf16 columns
        u = np.ascontiguousarray(raw).view(np.uint16)
        z = (u[:, C].astype(np.uint32)
             | (u[:, C + 1].astype(np.uint32) << 16)).view(np.float32)
        return raw[:, 0:C].astype(np.float32), z.reshape(-1, 1)

    out = np.empty((B, C, N), np.float32)
    bvf = np.asarray(bv, np.float32).reshape(1, C)
    for b in range(B):
        o0, z0 = _split(res.results[2 * b]["out"])
        o1, z1 = _split(res.results[2 * b + 1]["out"])
        o = o0 + np.roll(o1, MHALF, axis=0)  # un-rotate second key-half
        zz = z0 + np.roll(z1, MHALF, axis=0)
        out[b] = ((o / zz) + bvf).T
    return out.reshape(B, C, H, W)
